# revision 30
# baseline (speedup 1.0000x reference)
"""Trainium2 Bass kernel for nn_DWT_1D: db4 DWT along the last axis.

Reference computes lo = einsum('ncl,kl->nck', x, matrix_low) (and hi with
matrix_high); the matrices are banded+strided, so each output col k depends
on the 8 input elements x[2k-3 : 2k+5].

Strategy (data-parallel over N, 2 batch rows = 128 (n,c) rows per core):
  - The host transposes each core's input into NON-overlapping 128-element
    chunks laid out chunk-major and downcasts to fp16:
    xt[s, 128*c + p] = x[p, 128*c + s].  Output block b (64 cols per
    filter) is computed entirely in PSUM from chunk b via one K=128 fp16
    matmul against a banded weight block wmain[s,(f,r)] = rec_f[s-2r+3],
    plus two tiny seam matmuls that accumulate the filter taps falling in
    chunks b-1 / b+1 (start=False).  No on-device transposes, no overlap
    duplication, no padding; fp16 halves both PE time and HBM traffic
    vs fp32 (rel-err budget 2e-2; fp16 end-to-end lands ~3e-4).
  - 64 blocks in 16 PSUM groups of 4 (one full 2 KB bank each).  DVE copies
    even groups and ScalarE odd groups from PSUM (fp32) into fp16 SBUF
    slabs (cast on copy); slabs are DMA'd to HBM fp16; the host upcasts.
  - DMA engine occupancy is the roofline: ~2.14 MB in + ~2.10 MB out per
    core at 360 GB/s ~= 11.8 us.  Launches are spread across SP (input
    chunks + early slabs) and gpsimd SWDGE (last slab) so the transfer
    queue never starves.
"""

import numpy as np

import concourse.bacc as bacc
import concourse.bass as bass
import concourse.mybir as mybir
import concourse.tile as tile
from concourse.bass_utils import run_bass_kernel_spmd

FP16 = mybir.dt.float16
FP32 = mybir.dt.float32
P = 128
LIN = 8192
LOUT = 4096
NCORES = 8
BLKW = 64             # output cols per block per filter
NBLK = LOUT // BLKW   # 64
CPG = 4               # blocks per PSUM group
NGROUP = NBLK // CPG  # 16
GROUPW = CPG * BLKW   # 256 output cols per filter per group (1 PSUM bank)
WB = 144              # DMA'd weights: wmain 2*64 | wprev 2*2 | wnext 2*2 | pad
WCOLS = WB + LIN      # 8336
SEAM = 2              # seam cols on each side of a block boundary

# pywt db4 reconstruction filters (== wavelet.rec_lo / rec_hi used by the
# reference to build matrix_low / matrix_high; asserted against them at run)
REC_LO = (0.23037781330885523, 0.7148465705525415, 0.6308807679295904,
          -0.02798376941698385, -0.18703481171888114, 0.030841381835986965,
          0.032883011666982945, -0.010597401784997278)
REC_HI = (-0.010597401784997278, -0.032883011666982945, 0.030841381835986965,
          0.18703481171888114, -0.02798376941698385, -0.6308807679295904,
          0.7148465705525415, -0.23037781330885523)

# --- tunable schedule (see sweep.py) ---
CFG = dict(
    # input DMA split points (col indices into the combined [P, WCOLS] tensor)
    in_split=(0, 2192, 3728, 5264, 6800, WCOLS),
    # output slab boundaries in groups
    slab_g=(0, 3, 6, 10, 13, NGROUP),
    sp_slabs=4,       # slabs 0..sp_slabs-1 launched by SP, rest by gpsimd
    nwarm=2,          # dummy PE matmuls (no DMA deps) before the real ones
    warm_rows=512,    # rhs free size of each warm matmul
    copy_mod=2,       # group copies: g % copy_mod == 0 -> DVE, else ScalarE
    psum_bufs=7,
)

LAST_RESULTS = None   # BassKernelResults of the most recent run (for test.py)


def build_nc(cfg=CFG) -> bass.Bass:
    in_split = list(cfg["in_split"])
    slab_g = list(cfg["slab_g"])
    nslab = len(slab_g) - 1
    sp_slabs = cfg["sp_slabs"]
    nwarm = cfg["nwarm"]

    nc = bacc.Bacc("TRN2")
    xw = nc.dram_tensor("xw", [P, WCOLS], FP16, kind="ExternalInput")
    out = nc.dram_tensor("out", [P, 2, LOUT], FP16, kind="ExternalOutput")

    with tile.TileContext(nc) as tc:
        with (
            tc.tile_pool(name="xbuf", bufs=1) as xbuf_pool,
            tc.tile_pool(name="slab", bufs=1) as slab_pool,
            tc.tile_pool(name="wpsum", bufs=1, space="PSUM") as wpsum,
            tc.tile_pool(name="gpsum", bufs=cfg["psum_bufs"], space="PSUM") as gpsum,
        ):
            xw_sb = xbuf_pool.tile([P, WCOLS], FP16, tag="xw")
            for j in range(len(in_split) - 1):
                nc.sync.dma_start(
                    xw_sb[:, in_split[j] : in_split[j + 1]],
                    xw[:, in_split[j] : in_split[j + 1]],
                )

            # weight views from the DMA'd header cols [wmain | wprev | wnext]:
            # wmain[s, f, r] = rec_f[s - 2r + 3]   (taps inside chunk b)
            # wprev[s, f, r] = rec_f[s - 2r - 125] for r in {0, 1}
            # wnext[s, f, i] = rec_f[s - 2i + 5]   for i = r - 62 in {0, 1}
            wmain = xw_sb[:, 0 : 2 * BLKW].rearrange("p (f r) -> p f r", f=2)
            wprev = xw_sb[:, 2 * BLKW : 2 * BLKW + 4].rearrange(
                "p (f r) -> p f r", f=2
            )
            wnext = xw_sb[:, 2 * BLKW + 4 : 2 * BLKW + 8].rearrange(
                "p (f r) -> p f r", f=2
            )

            def chunk(c):
                a = WB + P * c
                return xw_sb[:, a : a + P]

            if nwarm:
                # keep the PE saturated while the first input DMAs land:
                # dummy matmuls on a locally memset scratch tile
                wr = cfg["warm_rows"]
                warm_sb = xbuf_pool.tile([P, wr], FP16, tag="warm_sb")
                nc.vector.memset(warm_sb[:], 0.0)
                warm_ps = wpsum.tile([P, wr], FP32, tag="warm", bufs=1)
                for _ in range(nwarm):
                    nc.tensor.matmul(
                        warm_ps[:], warm_sb[:, :P], warm_sb[:], start=True, stop=True
                    )

            slabs = [None] * nslab
            slab_of = {}
            for m in range(nslab):
                for g in range(slab_g[m], slab_g[m + 1]):
                    slab_of[g] = m

            def slab_cols(m):
                return slab_g[m] * GROUPW, slab_g[m + 1] * GROUPW

            for g in range(NGROUP):
                gt = gpsum.tile([P, 2, GROUPW], FP32, tag="gt", name=f"gt{g}")
                for i in range(CPG):
                    b = CPG * g + i
                    o = BLKW * i
                    # main: all taps of block b that live in chunk b
                    nc.tensor.matmul(
                        gt[:, :, o : o + BLKW], chunk(b), wmain[:, :, :BLKW],
                        start=True, stop=False, skip_group_check=True,
                    )
                    # seam taps from chunk b-1 (first 2 out cols) and
                    # chunk b+1 (last 2 out cols), accumulated in PSUM
                    if b > 0:
                        nc.tensor.matmul(
                            gt[:, :, o : o + SEAM], chunk(b - 1), wprev[:, :, :],
                            start=False, stop=False, skip_group_check=True,
                        )
                    if b < NBLK - 1:
                        nc.tensor.matmul(
                            gt[:, :, o + BLKW - SEAM : o + BLKW],
                            chunk(b + 1), wnext[:, :, :],
                            start=False, stop=True, skip_group_check=True,
                        )
                m = slab_of[g]
                c0, c1 = slab_cols(m)
                if slabs[m] is None:
                    slabs[m] = slab_pool.tile(
                        [P, 2, c1 - c0], FP16, tag=f"slab{m}", name=f"slab{m}"
                    )
                off = g * GROUPW - c0
                if g % cfg["copy_mod"] == 0:
                    nc.vector.tensor_copy(
                        slabs[m][:, :, off : off + GROUPW], gt[:, :, :]
                    )
                else:
                    nc.scalar.copy(slabs[m][:, :, off : off + GROUPW], gt[:, :, :])
                if g == slab_g[m + 1] - 1:
                    if m < sp_slabs:
                        nc.sync.dma_start(out[:, :, c0:c1], slabs[m][:])
                    else:
                        # gpsimd SWDGE: extra launch lane, keeps the queue
                        # fed while SP is still launching earlier slabs
                        nc.gpsimd.dma_start(out[:, :, c0:c1], slabs[m][:])
    nc.compile()
    return nc


_NC_CACHE = None


def _get_nc() -> bass.Bass:
    global _NC_CACHE
    if _NC_CACHE is None:
        _NC_CACHE = build_nc()
    return _NC_CACHE


def kernel(input, matrix_low, matrix_high, *, trace=False, tmpdir=None):
    global LAST_RESULTS
    x = np.ascontiguousarray(np.asarray(input, dtype=np.float32))
    ml = np.asarray(matrix_low, dtype=np.float32)
    mh = np.asarray(matrix_high, dtype=np.float32)
    assert x.shape == (16, 64, LIN), x.shape

    # The weight header is derived from interior matrix rows assuming the
    # band is shift-invariant: M[k, l] = rec[l - 2k + 3].  Guard that
    # assumption (a mismatch would otherwise produce silently wrong output).
    assert np.allclose(ml[64, 125:133], REC_LO, atol=1e-7), ml[64, 125:133]
    assert np.allclose(mh[64, 125:133], REC_HI, atol=1e-7), mh[64, 125:133]

    # DMA'd weight header (144 cols), derived from interior matrix rows
    # (shift-invariant): wmain[s, f, r] = M_f[64+r, 128+s],
    # wprev[s, f, r] = M_f[64+r, s], wnext[s, f, i] = M_f[126+i, 256+s].
    whdr = np.zeros((P, WB), dtype=np.float32)
    wm = np.zeros((P, 2, BLKW), dtype=np.float32)
    wp = np.zeros((P, 2, SEAM), dtype=np.float32)
    wn = np.zeros((P, 2, SEAM), dtype=np.float32)
    for f, M in enumerate((ml, mh)):
        wm[:, f, :] = M[64 : 64 + BLKW, 128 : 128 + P].T
        wp[:, f, :] = M[64 : 64 + SEAM, 0:P].T
        wn[:, f, :] = M[64 + BLKW - SEAM : 64 + BLKW, 256 : 256 + P].T
    whdr[:, 0 : 2 * BLKW] = wm.reshape(P, 2 * BLKW)
    whdr[:, 2 * BLKW : 2 * BLKW + 4] = wp.reshape(P, 4)
    whdr[:, 2 * BLKW + 4 : 2 * BLKW + 8] = wn.reshape(P, 4)
    whdr_h = whdr.astype(np.float16)

    nc = _get_nc()
    in_maps = []
    for d in range(NCORES):
        xc = x[2 * d : 2 * d + 2].reshape(P, LIN)
        # xt[s, 128 c + p] = x[p, 128 c + s]
        xt = np.ascontiguousarray(
            xc.reshape(P, NBLK, P).transpose(2, 1, 0)
        ).reshape(P, LIN)
        xwa = np.empty((P, WCOLS), dtype=np.float16)
        xwa[:, :WB] = whdr_h
        xwa[:, WB:] = xt.astype(np.float16)
        in_maps.append({"xw": xwa})

    res = run_bass_kernel_spmd(
        nc, in_maps, core_ids=list(range(NCORES)), trace=trace, tmpdir=tmpdir
    )
    LAST_RESULTS = res
    both = np.stack([r["out"] for r in res.results])     # (8, 128, 2, LOUT) fp16
    both = both.astype(np.float32).reshape(NCORES, 2, 64, 2, LOUT)
    lo = np.ascontiguousarray(both[:, :, :, 0, :].reshape(16, 64, LOUT))
    hi = np.ascontiguousarray(both[:, :, :, 1, :].reshape(16, 64, LOUT))
    return lo, hi
